# revision 14
# baseline (speedup 1.0000x reference)
"""Adagnn-with-weight GNN message-passing kernel for 8 Trainium2 NeuronCores.

Reference computation (N=100000 nodes, E=3200000 edges, F=256):
    e1  = segment_sum(edge_val[:,None] * x[edge_col], edge_row)   # spmm
    out = (x - e1 * (learnable_diag + 1)) @ weight + bias

Strategy (data-parallel over destination nodes, no collectives):
  - Host: partition edges by destination core (12500 rows each). Within a
    core, order its 98 dest tiles by edge count (descending) so that the
    same slot on all 8 cores holds similarly-sized tiles (the compiled
    graph is shared; per-cell gather sizes are maxes over cores, so
    aligning sizes minimizes padding). Sort edges by (slot, source block
    of 25000 rows), pad each (slot, block) cell to a multiple of 128
    (chunk) for the PE, but gather only roundup16(max-over-cores count)
    rows (pad indices are 0 -> they fetch block row 0; their A-column
    weight is 0 so they contribute nothing; slots beyond num_idxs keep
    stale-but-finite data from a one-time warming memset).
  - Device, per dest slot s:
      * dma_gather x rows (bf16, int16 idx < 25000) for each source block
        into column ranges of ONE per-slot buffer xg[128, n_t, 256];
        queue = block index so each SWDGE queue walks one sorted 12.8MB
        HBM window.
      * one-hot A[e, d, c] = (iota[d] == dest[e,c]) * val[e,c] built with
        two whole-tile DVE tensor_tensor passes in [e, d, c] layout:
        dest/val broadcast along the MIDDLE dim and a materialized
        iota_rep constant keep every operand's innermost AP dim stride-1,
        which qualifies the bf16 passes for the DVE 2x perf mode
        (innermost stride-0 broadcasts force 1x).
      * PE: e1[128 dest, 256] += A_c^T @ Xg_c over the slot's chunks.
      * epilogue (software-pipelined one slot behind the segment-sum so PE
        never waits on DVE): e4 = xo - e1*(diag+1) on DVE; transpose e4 on
        PE; out_tile = e4 @ W + bias on PE/DVE; DMA out in bf16.
"""

import numpy as np

import concourse.bacc as bacc
import concourse.mybir as mybir
import concourse.tile as tile
from concourse.bass_utils import run_bass_kernel_spmd

FP = mybir.dt.float32
BF = mybir.dt.bfloat16
BF_NP = mybir.dt.np(BF)


class Cfg:
    def __init__(self, n_nodes=100000, n_edges=3200000, f=256, n_cores=8,
                 nb=4, gather_bufs=2, amat_bufs=3, gs=3):
        assert n_nodes % (n_cores * nb) == 0
        self.N = n_nodes
        self.E = n_edges
        self.F = f
        self.NC = n_cores
        self.NB = nb
        self.RPC = n_nodes // n_cores
        self.TILES = (self.RPC + 127) // 128
        self.PAD_ROWS = self.TILES * 128
        self.BLK = n_nodes // nb
        assert self.BLK < (1 << 15)
        self.gather_bufs = gather_bufs   # per-(group, block) buffers
        self.amat_bufs = amat_bufs
        self.GS = gs


def _preprocess(cfg, edge_row, edge_col, edge_val):
    """Partition + balance + sort + pad the edge list."""
    edge_row = np.asarray(edge_row).astype(np.int64)
    edge_col = np.asarray(edge_col).astype(np.int64)
    edge_val = np.asarray(edge_val).astype(np.float32)
    NC, TILES, NB, E = cfg.NC, cfg.TILES, cfg.NB, cfg.E

    core = edge_row // cfg.RPC
    dloc = edge_row - core * cfg.RPC
    t = dloc >> 7
    d = (dloc & 127).astype(np.float32)
    b = edge_col // cfg.BLK
    cloc = (edge_col - b * cfg.BLK).astype(np.int16)

    # per-core tile totals -> slot ordering (largest tile first on every
    # core, so cell sizes line up across cores)
    tile_tot = np.zeros((NC, TILES), dtype=np.int64)
    np.add.at(tile_tot, (core, t), 1)
    tile_at = np.argsort(-tile_tot, axis=1, kind="stable")   # [NC, slot]->tile
    slot_of = np.empty_like(tile_at)
    rows = np.arange(TILES)[None, :].repeat(NC, 0)
    np.put_along_axis(slot_of, tile_at, rows, axis=1)        # [NC, tile]->slot

    s = slot_of[core, t]
    ncell = TILES * NB
    # cell order for the A/dv (per-slot, block-major) tables: (s, b)
    key = core * ncell + s * NB + b
    order = np.lexsort((cloc, key))
    key_s = key[order]

    counts = np.bincount(key, minlength=NC * ncell).reshape(NC, ncell)
    maxcnt = counts.max(axis=0)                              # [ncell]
    C = np.ceil(maxcnt / 128).astype(np.int64)               # chunks per cell
    NIDX = (np.ceil(maxcnt / 16) * 16).astype(np.int64)      # gathered rows
    pad_off = np.concatenate([[0], np.cumsum(128 * C)])      # [ncell+1]
    L = int(pad_off[-1])

    # idx/xg slot layout groups GS consecutive dest slots and orders cells
    # (group, block, slot-in-group) so one gather covers a whole
    # (group, block) range
    GS = cfg.GS
    cs, cb_ = np.divmod(np.arange(ncell), NB)                # cell -> (s, b)
    gkey = (cs // GS) * (NB * GS) + cb_ * GS + (cs % GS)     # cell -> g-rank
    gorder = np.argsort(gkey)                                # g-rank -> cell
    goff = np.concatenate([[0], np.cumsum(128 * C[gorder])])
    cell_goff = np.empty(ncell, dtype=np.int64)              # cell -> idx off
    cell_goff[gorder] = goff[:-1]

    starts = np.searchsorted(key_s, np.arange(NC * ncell), side="left")
    rank = np.arange(E) - starts[key_s]
    pos = (key_s // ncell) * L + pad_off[key_s % ncell] + rank
    posg = (key_s // ncell) * L + cell_goff[key_s % ncell] + rank

    col_pad = np.zeros(NC * L, dtype=np.int16)               # pads -> row 0
    dest_pad = np.zeros(NC * L, dtype=np.float32)
    val_pad = np.zeros(NC * L, dtype=np.float32)             # pads -> A==0
    col_pad[posg] = cloc[order]                              # grouped layout
    dest_pad[pos] = d[order]
    val_pad[pos] = edge_val[order]

    col_pad = col_pad.reshape(NC, L)
    dest_pad = dest_pad.reshape(NC, L)
    val_pad = val_pad.reshape(NC, L)

    # wrapped int16 index layout: element i -> [i % 16, i // 16], x8 replicate
    idx_packed = np.tile(
        col_pad.reshape(NC, L // 16, 16).transpose(0, 2, 1), (1, 8, 1)
    )  # [NC, 128, L//16]

    # per-slot [128, 2, C_t] merged dest/val layout (chunk-major columns)
    C2 = C.reshape(TILES, NB)
    Ct = C2.sum(axis=1)                                      # [TILES]
    CT = int(Ct.sum())
    dv_cols = np.zeros((NC, 128, 2, CT), dtype=BF_NP)
    toff = 0
    for ss in range(TILES):
        o0 = int(pad_off[ss * NB])
        n = int(Ct[ss])
        seg = slice(o0, o0 + 128 * n)
        dv_cols[:, :, 0, toff:toff + n] = (
            dest_pad[:, seg].reshape(NC, n, 128).transpose(0, 2, 1))
        dv_cols[:, :, 1, toff:toff + n] = (
            val_pad[:, seg].reshape(NC, n, 128).transpose(0, 2, 1))
        toff += n

    NIDX2 = NIDX.reshape(TILES, NB)
    cell_goff2 = cell_goff.reshape(TILES, NB)
    return C2, NIDX2, pad_off, cell_goff2, idx_packed, dv_cols, tile_at


def _build(cfg, C2, NIDX2, pad_off, cell_goff2):
    """Build the (shared) per-core Bass graph given the chunk table."""
    F, NB, TILES = cfg.F, cfg.NB, cfg.TILES
    GS = cfg.GS
    Ct = C2.sum(axis=1)
    CT = int(Ct.sum())
    L = int(pad_off[-1])
    C_MAXT = int(Ct.max())
    NG = (TILES + GS - 1) // GS
    # per-(group, block) chunk counts
    CGB = np.zeros((NG, NB), dtype=np.int64)
    for g in range(NG):
        CGB[g] = C2[g * GS:(g + 1) * GS].sum(axis=0)
    C_MAXGB = int(CGB.max())
    KC = F // 128

    nc = bacc.Bacc("TRN2", target_bir_lowering=False, debug=False,
                   num_swdge_queues=4)

    xsrc = nc.dram_tensor("xsrc", [cfg.N, F], BF, kind="ExternalInput")
    xown = nc.dram_tensor("xown", [cfg.PAD_ROWS, F], BF, kind="ExternalInput")
    idx_d = nc.dram_tensor("idx", [128, L // 16], mybir.dt.int16,
                           kind="ExternalInput")
    dv_d = nc.dram_tensor("dv", [128, 2, CT], BF, kind="ExternalInput")
    w_d = nc.dram_tensor("wt", [128, KC, F], BF, kind="ExternalInput")
    dscale_d = nc.dram_tensor("dscale", [128, F], FP, kind="ExternalInput")
    bias_d = nc.dram_tensor("bias", [128, F], FP, kind="ExternalInput")
    iota_d = nc.dram_tensor("iota", [128, 128, C_MAXT], BF,
                            kind="ExternalInput")
    ident_d = nc.dram_tensor("ident", [128, 128], BF, kind="ExternalInput")
    out_d = nc.dram_tensor("out", [cfg.PAD_ROWS, F], BF, kind="ExternalOutput")

    with tile.TileContext(nc) as tc:
        with (
            tc.tile_pool(name="const", bufs=1) as cpool,
            tc.tile_pool(name="gather", bufs=cfg.gather_bufs * cfg.NB) as gpool,
            tc.tile_pool(name="amat", bufs=cfg.amat_bufs) as apool,
            tc.tile_pool(name="meta", bufs=6) as mpool,
            tc.tile_pool(name="work", bufs=3) as wpool,
            tc.tile_pool(name="pse1", bufs=2, space="PSUM") as e1pool,
            tc.tile_pool(name="pstr", bufs=2, space="PSUM") as trpool,
            tc.tile_pool(name="psout", bufs=2, space="PSUM") as opool,
        ):
            w_t = cpool.tile([128, KC, F], BF)
            dscale_t = cpool.tile([128, F], FP)
            bias_t = cpool.tile([128, F], FP)
            iota_t = cpool.tile([128, 128, C_MAXT], BF)
            ident_t = cpool.tile([128, 128], BF)
            nc.sync.dma_start(w_t[:], w_d[:])
            nc.sync.dma_start(dscale_t[:], dscale_d[:])
            nc.sync.dma_start(bias_t[:], bias_d[:])
            nc.sync.dma_start(iota_t[:], iota_d[:])
            nc.sync.dma_start(ident_t[:], ident_d[:])
            # warm every gather slot: pads inside a group are gathered (row
            # 0) but the graph is shared across cores, so slots past a
            # core's true counts hold stale data; it must be finite (the
            # A-column is 0)
            for _ in range(cfg.gather_bufs * NB):
                xg_w = gpool.tile([128, C_MAXGB, F], BF, tag="xg")
                nc.vector.memset(xg_w[:], 0.0)

            toff = 0
            prev = None          # (e1, xo, slot) of the previous slot
            xgs = {}             # block -> current group's gather buffer
            for tt in range(TILES):
                n_t = int(Ct[tt])

                if tt % GS == 0:
                    # one gather per source block covering GS slots' cells
                    g = tt // GS
                    idxg = mpool.tile(
                        [128, 8 * int(CGB[g].sum())], mybir.dt.int16,
                        tag="idx")
                    o16 = int(cell_goff2[tt, 0]) // 16
                    nc.sync.dma_start(
                        idxg[:],
                        idx_d[:, o16:o16 + 8 * int(CGB[g].sum())])
                    io = 0
                    for bb in range(NB):
                        cgb = int(CGB[g, bb])
                        if cgb == 0:
                            continue
                        xg = gpool.tile([128, C_MAXGB, F], BF, tag="xg")
                        nc.gpsimd.dma_gather(
                            xg[:, :cgb, :],
                            xsrc[bb * cfg.BLK:(bb + 1) * cfg.BLK, :],
                            idxg[:, 8 * io:8 * (io + cgb)],
                            num_idxs=128 * cgb,
                            num_idxs_reg=128 * cgb,
                            elem_size=F,
                            single_packet=False,
                            queue_num=bb,
                        )
                        xgs[bb] = (xg, [int(x) for x in np.concatenate(
                            [[0], np.cumsum(C2[g * GS:(g + 1) * GS, bb])])])
                        io += cgb

                dv_t = mpool.tile([128, 2, n_t], BF, tag="dv")
                nc.sync.dma_start(dv_t[:], dv_d[:, :, toff:toff + n_t])
                xo = wpool.tile([128, F], BF, tag="xo")
                nc.sync.dma_start(xo[:], xown[tt * 128:(tt + 1) * 128, :])

                # one-hot A in [e, d, c] layout: dest/val broadcast on the
                # middle dim; all innermost dims stride-1 -> DVE 2x eligible
                a_t = apool.tile([128, 128, C_MAXT], BF, tag="a")
                dest_b = dv_t[:, 0, None, :].broadcast_to((128, 128, n_t))
                val_b = dv_t[:, 1, None, :].broadcast_to((128, 128, n_t))
                nc.vector.tensor_tensor(a_t[:, :, :n_t], iota_t[:, :, :n_t],
                                        dest_b, op=mybir.AluOpType.is_equal)
                nc.vector.tensor_tensor(a_t[:, :, :n_t], a_t[:, :, :n_t],
                                        val_b, op=mybir.AluOpType.mult)

                # segment-sum into PSUM; chunk c of this slot lives in the
                # per-(group, block) buffer at its slot's offset
                e1 = e1pool.tile([128, F], FP, tag="e1")
                sg = tt % GS
                cc = 0
                for bb in range(NB):
                    cb = int(C2[tt, bb])
                    if cb == 0:
                        continue
                    xgb, offs = xgs[bb]
                    o = offs[sg]
                    for c in range(cb):
                        nc.tensor.matmul(
                            e1[:], a_t[:, :, cc], xgb[:, o + c, :],
                            start=(cc == 0), stop=(cc == n_t - 1),
                        )
                        cc += 1

                # epilogue of the PREVIOUS slot (so PE goes straight from
                # this slot's segment-sum into finished work)
                if prev is not None:
                    _epilogue(nc, cfg, prev, wpool, trpool, opool,
                              dscale_t, bias_t, ident_t, w_t, out_d)
                prev = (e1, xo, tt)
                toff += n_t

            _epilogue(nc, cfg, prev, wpool, trpool, opool,
                      dscale_t, bias_t, ident_t, w_t, out_d)

    nc.compile()
    return nc


def _epilogue(nc, cfg, prev, wpool, trpool, opool,
              dscale_t, bias_t, ident_t, w_t, out_d):
    F = cfg.F
    KC = F // 128
    e1, xo, tt = prev
    # e4 = xo - e1 * dscale   (bf16 result for the projection)
    t0 = wpool.tile([128, F], FP, tag="t0")
    nc.vector.tensor_tensor(t0[:], e1[:], dscale_t[:],
                            op=mybir.AluOpType.mult)
    e4 = wpool.tile([128, F], BF, tag="e4")
    nc.vector.tensor_tensor(e4[:], xo[:], t0[:],
                            op=mybir.AluOpType.subtract)

    # transpose e4 (PE), copy to SBUF on ACT
    ps_tr = trpool.tile([128, KC, 128], BF, tag="tr")
    for kc in range(KC):
        nc.tensor.transpose(ps_tr[:, kc, :],
                            e4[:, kc * 128:(kc + 1) * 128],
                            ident_t[:])
    e4T = wpool.tile([128, KC, 128], BF, tag="e4T")
    nc.scalar.copy(e4T[:], ps_tr[:])

    # out = e4 @ W + bias
    ps_out = opool.tile([128, F], FP, tag="po")
    for kc in range(KC):
        nc.tensor.matmul(ps_out[:], e4T[:, kc, :], w_t[:, kc, :],
                         start=(kc == 0), stop=(kc == KC - 1))
    outs = wpool.tile([128, F], BF, tag="outs")
    nc.vector.tensor_tensor(outs[:], ps_out[:], bias_t[:],
                            op=mybir.AluOpType.add)
    nc.sync.dma_start(out_d[tt * 128:(tt + 1) * 128, :], outs[:])


def _make_in_maps(cfg, c_maxt, x, weight, learnable_diag, bias,
                  idx_packed, dv_cols, tile_at):
    F, NC = cfg.F, cfg.NC
    x16 = x.astype(BF_NP)
    w_host = np.ascontiguousarray(
        weight.reshape(F // 128, 128, F).transpose(1, 0, 2)).astype(BF_NP)
    dscale_host = np.tile((learnable_diag + 1.0)[None, :], (128, 1))
    bias_host = np.tile(bias[None, :], (128, 1))
    iota_host = np.ascontiguousarray(np.broadcast_to(
        np.arange(128, dtype=np.float32).astype(BF_NP)[None, :, None],
        (128, 128, c_maxt)))
    ident_host = np.eye(128, dtype=np.float32).astype(BF_NP)

    # xown reordered by slot: slot s of core c holds tile tile_at[c, s]
    xpad = np.zeros((NC, cfg.PAD_ROWS, F), dtype=np.float32)
    xpad[:, :cfg.RPC, :] = x.reshape(NC, cfg.RPC, F)
    xown_slot = np.empty((NC, cfg.PAD_ROWS, F), dtype=BF_NP)
    for c in range(NC):
        xown_slot[c] = xpad[c].reshape(cfg.TILES, 128, F)[
            tile_at[c]].reshape(cfg.PAD_ROWS, F)

    in_maps = []
    for c in range(NC):
        in_maps.append({
            "xsrc": x16,
            "xown": xown_slot[c],
            "idx": np.ascontiguousarray(idx_packed[c]),
            "dv": np.ascontiguousarray(dv_cols[c]),
            "wt": w_host,
            "dscale": dscale_host,
            "bias": bias_host,
            "iota": iota_host,
            "ident": ident_host,
        })
    return in_maps


def run(cfg, x, edge_row, edge_col, edge_val, weight, learnable_diag, bias,
        trace_dir=None):
    x = np.ascontiguousarray(np.asarray(x, dtype=np.float32))
    weight = np.asarray(weight, dtype=np.float32)
    learnable_diag = np.asarray(learnable_diag, dtype=np.float32)
    bias = np.asarray(bias, dtype=np.float32)

    C2, NIDX2, pad_off, cell_goff2, idx_packed, dv_cols, tile_at = \
        _preprocess(cfg, edge_row, edge_col, edge_val)
    nc = _build(cfg, C2, NIDX2, pad_off, cell_goff2)
    c_maxt = int(C2.sum(axis=1).max())
    in_maps = _make_in_maps(cfg, c_maxt, x, weight, learnable_diag, bias,
                            idx_packed, dv_cols, tile_at)

    kwargs = {}
    if trace_dir:
        kwargs = dict(trace=True, tmpdir=trace_dir)
    res = run_bass_kernel_spmd(nc, in_maps, core_ids=list(range(cfg.NC)),
                               **kwargs)
    out = np.empty((cfg.N, cfg.F), dtype=np.float32)
    for c in range(cfg.NC):
        o = res.results[c]["out"].astype(np.float32).reshape(
            cfg.TILES, 128, cfg.F)
        full = np.empty((cfg.TILES, 128, cfg.F), dtype=np.float32)
        full[tile_at[c]] = o                     # slot s -> tile tile_at[c,s]
        out[c * cfg.RPC:(c + 1) * cfg.RPC] = full.reshape(
            cfg.PAD_ROWS, cfg.F)[:cfg.RPC]
    return out, res


def kernel(x, edge_row, edge_col, edge_val, weight, learnable_diag, bias,
           _want_trace=None):
    cfg = Cfg()
    out, res = run(cfg, x, edge_row, edge_col, edge_val, weight,
                   learnable_diag, bias, trace_dir=_want_trace)
    kernel._last_results = res
    return out


# revision 15
# speedup vs baseline: 1.1506x; 1.1506x over previous
"""Adagnn-with-weight GNN message-passing kernel for 8 Trainium2 NeuronCores.

Reference computation (N=100000 nodes, E=3200000 edges, F=256):
    e1  = segment_sum(edge_val[:,None] * x[edge_col], edge_row)   # spmm
    out = (x - e1 * (learnable_diag + 1)) @ weight + bias

Strategy (data-parallel over destination nodes, no collectives):
  - Host: partition edges by destination core (12500 rows each). Within a
    core, order its 98 dest tiles by edge count (descending) so that the
    same slot on all 8 cores holds similarly-sized tiles (the compiled
    graph is shared; per-cell gather sizes are maxes over cores, so
    aligning sizes minimizes padding). Sort edges by (slot, source block
    of 25000 rows), pad each (slot, block) cell to a multiple of 128
    (chunk) for the PE, but gather only roundup16(max-over-cores count)
    rows (pad indices are 0 -> they fetch block row 0; their A-column
    weight is 0 so they contribute nothing; slots beyond num_idxs keep
    stale-but-finite data from a one-time warming memset).
  - Device, per dest slot s:
      * dma_gather x rows (bf16, int16 idx < 25000) for each source block
        into column ranges of ONE per-slot buffer xg[128, n_t, 256];
        queue = block index so each SWDGE queue walks one sorted 12.8MB
        HBM window.
      * one-hot A[e, d, c] = (iota[d] == dest[e,c]) * val[e,c] built with
        two whole-tile DVE tensor_tensor passes in [e, d, c] layout:
        dest/val broadcast along the MIDDLE dim and a materialized
        iota_rep constant keep every operand's innermost AP dim stride-1,
        which qualifies the bf16 passes for the DVE 2x perf mode
        (innermost stride-0 broadcasts force 1x).
      * PE: e1[128 dest, 256] += A_c^T @ Xg_c over the slot's chunks.
      * epilogue (software-pipelined one slot behind the segment-sum so PE
        never waits on DVE): e4 = xo - e1*(diag+1) on DVE; transpose e4 on
        PE; out_tile = e4 @ W + bias on PE/DVE; DMA out in bf16.
"""

import numpy as np

import concourse.bacc as bacc
import concourse.mybir as mybir
import concourse.tile as tile
from concourse.bass_utils import run_bass_kernel_spmd

FP = mybir.dt.float32
BF = mybir.dt.bfloat16
BF_NP = mybir.dt.np(BF)


class Cfg:
    def __init__(self, n_nodes=100000, n_edges=3200000, f=256, n_cores=8,
                 nb=4, gather_bufs=7, amat_bufs=3):
        assert n_nodes % (n_cores * nb) == 0
        self.N = n_nodes
        self.E = n_edges
        self.F = f
        self.NC = n_cores
        self.NB = nb
        self.RPC = n_nodes // n_cores
        self.TILES = (self.RPC + 127) // 128
        self.PAD_ROWS = self.TILES * 128
        self.BLK = n_nodes // nb
        assert self.BLK < (1 << 15)
        self.gather_bufs = gather_bufs
        self.amat_bufs = amat_bufs


def _preprocess(cfg, edge_row, edge_col, edge_val):
    """Partition + balance + sort + pad the edge list."""
    edge_row = np.asarray(edge_row).astype(np.int64)
    edge_col = np.asarray(edge_col).astype(np.int64)
    edge_val = np.asarray(edge_val).astype(np.float32)
    NC, TILES, NB, E = cfg.NC, cfg.TILES, cfg.NB, cfg.E

    core = edge_row // cfg.RPC
    dloc = edge_row - core * cfg.RPC
    t = dloc >> 7
    d = (dloc & 127).astype(np.float32)
    b = edge_col // cfg.BLK
    cloc = (edge_col - b * cfg.BLK).astype(np.int16)

    # per-core tile totals -> slot ordering (largest tile first on every
    # core, so cell sizes line up across cores)
    tile_tot = np.zeros((NC, TILES), dtype=np.int64)
    np.add.at(tile_tot, (core, t), 1)
    tile_at = np.argsort(-tile_tot, axis=1, kind="stable")   # [NC, slot]->tile
    slot_of = np.empty_like(tile_at)
    rows = np.arange(TILES)[None, :].repeat(NC, 0)
    np.put_along_axis(slot_of, tile_at, rows, axis=1)        # [NC, tile]->slot

    s = slot_of[core, t]
    ncell = TILES * NB
    # cell order for the A/dv (per-slot, block-major) tables: (s, b)
    key = core * ncell + s * NB + b
    order = np.lexsort((cloc, key))
    key_s = key[order]

    counts = np.bincount(key, minlength=NC * ncell).reshape(NC, ncell)
    maxcnt = counts.max(axis=0)                              # [ncell]
    C = np.ceil(maxcnt / 128).astype(np.int64)               # chunks per cell
    NIDX = (np.ceil(maxcnt / 16) * 16).astype(np.int64)      # gathered rows
    pad_off = np.concatenate([[0], np.cumsum(128 * C)])      # [ncell+1]
    L = int(pad_off[-1])

    starts = np.searchsorted(key_s, np.arange(NC * ncell), side="left")
    rank = np.arange(E) - starts[key_s]
    pos = (key_s // ncell) * L + pad_off[key_s % ncell] + rank

    col_pad = np.zeros(NC * L, dtype=np.int16)               # pads -> row 0
    dest_pad = np.zeros(NC * L, dtype=np.float32)
    val_pad = np.zeros(NC * L, dtype=np.float32)             # pads -> A==0
    col_pad[pos] = cloc[order]
    dest_pad[pos] = d[order]
    val_pad[pos] = edge_val[order]

    col_pad = col_pad.reshape(NC, L)
    dest_pad = dest_pad.reshape(NC, L)
    val_pad = val_pad.reshape(NC, L)

    # wrapped int16 index layout: element i -> [i % 16, i // 16], x8 replicate
    idx_packed = np.tile(
        col_pad.reshape(NC, L // 16, 16).transpose(0, 2, 1), (1, 8, 1)
    )  # [NC, 128, L//16]

    # per-slot [128, 2, C_t] merged dest/val layout (chunk-major columns)
    C2 = C.reshape(TILES, NB)
    Ct = C2.sum(axis=1)                                      # [TILES]
    CT = int(Ct.sum())
    dv_cols = np.zeros((NC, 128, 2, CT), dtype=BF_NP)
    toff = 0
    for ss in range(TILES):
        o0 = int(pad_off[ss * NB])
        n = int(Ct[ss])
        seg = slice(o0, o0 + 128 * n)
        dv_cols[:, :, 0, toff:toff + n] = (
            dest_pad[:, seg].reshape(NC, n, 128).transpose(0, 2, 1))
        dv_cols[:, :, 1, toff:toff + n] = (
            val_pad[:, seg].reshape(NC, n, 128).transpose(0, 2, 1))
        toff += n

    NIDX2 = NIDX.reshape(TILES, NB)
    return C2, NIDX2, pad_off, idx_packed, dv_cols, tile_at


def _build(cfg, C2, NIDX2, pad_off):
    """Build the (shared) per-core Bass graph given the chunk table."""
    F, NB, TILES = cfg.F, cfg.NB, cfg.TILES
    Ct = C2.sum(axis=1)
    CT = int(Ct.sum())
    L = int(pad_off[-1])
    C_MAXT = int(Ct.max())
    KC = F // 128

    nc = bacc.Bacc("TRN2", target_bir_lowering=False, debug=False,
                   num_swdge_queues=4)

    xsrc = nc.dram_tensor("xsrc", [cfg.N, F], BF, kind="ExternalInput")
    xown = nc.dram_tensor("xown", [cfg.PAD_ROWS, F], BF, kind="ExternalInput")
    idx_d = nc.dram_tensor("idx", [128, L // 16], mybir.dt.int16,
                           kind="ExternalInput")
    dv_d = nc.dram_tensor("dv", [128, 2, CT], BF, kind="ExternalInput")
    w_d = nc.dram_tensor("wt", [128, KC, F], BF, kind="ExternalInput")
    dscale_d = nc.dram_tensor("dscale", [128, F], FP, kind="ExternalInput")
    bias_d = nc.dram_tensor("bias", [128, F], FP, kind="ExternalInput")
    iota_d = nc.dram_tensor("iota", [128, 128, C_MAXT], BF,
                            kind="ExternalInput")
    ident_d = nc.dram_tensor("ident", [128, 128], BF, kind="ExternalInput")
    out_d = nc.dram_tensor("out", [cfg.PAD_ROWS, F], BF, kind="ExternalOutput")

    with tile.TileContext(nc) as tc:
        with (
            tc.tile_pool(name="const", bufs=1) as cpool,
            tc.tile_pool(name="gather", bufs=cfg.gather_bufs) as gpool,
            tc.tile_pool(name="amat", bufs=cfg.amat_bufs) as apool,
            tc.tile_pool(name="meta", bufs=6) as mpool,
            tc.tile_pool(name="work", bufs=3) as wpool,
            tc.tile_pool(name="pse1", bufs=2, space="PSUM") as e1pool,
            tc.tile_pool(name="pstr", bufs=2, space="PSUM") as trpool,
            tc.tile_pool(name="psout", bufs=2, space="PSUM") as opool,
        ):
            w_t = cpool.tile([128, KC, F], BF)
            dscale_t = cpool.tile([128, F], FP)
            bias_t = cpool.tile([128, F], FP)
            iota_t = cpool.tile([128, 128, C_MAXT], BF)
            ident_t = cpool.tile([128, 128], BF)
            nc.sync.dma_start(w_t[:], w_d[:])
            nc.sync.dma_start(dscale_t[:], dscale_d[:])
            nc.sync.dma_start(bias_t[:], bias_d[:])
            nc.sync.dma_start(iota_t[:], iota_d[:])
            nc.sync.dma_start(ident_t[:], ident_d[:])
            # warm every gather slot: num_idxs < 128*cb leaves tail slots of
            # the last chunk unwritten; stale data must be finite (A-col is 0)
            for _ in range(cfg.gather_bufs):
                xg_w = gpool.tile([128, C_MAXT, F], BF, tag="xg")
                nc.vector.memset(xg_w[:], 0.0)

            toff = 0
            prev = None          # (e1, xo, slot) of the previous slot
            for tt in range(TILES):
                n_t = int(Ct[tt])
                o16 = int(pad_off[tt * NB]) // 16

                idx_t = mpool.tile([128, 8 * n_t], mybir.dt.int16, tag="idx")
                nc.sync.dma_start(idx_t[:], idx_d[:, o16:o16 + 8 * n_t])
                dv_t = mpool.tile([128, 2, n_t], BF, tag="dv")
                nc.sync.dma_start(dv_t[:], dv_d[:, :, toff:toff + n_t])
                xo = wpool.tile([128, F], BF, tag="xo")
                nc.sync.dma_start(xo[:], xown[tt * 128:(tt + 1) * 128, :])

                # gathers: one per non-empty source block, all into one
                # per-slot buffer (disjoint column ranges)
                xg = gpool.tile([128, C_MAXT, F], BF, tag="xg")
                coffs = np.concatenate([[0], np.cumsum(C2[tt])]).astype(int)
                for i in range(NB):
                    bb = (i + tt) % NB       # rotate issue order per tile
                    cb = int(C2[tt, bb])
                    if cb == 0:
                        continue
                    coff = int(coffs[bb])
                    nidx = int(NIDX2[tt, bb])
                    nc.gpsimd.dma_gather(
                        xg[:, coff:coff + cb, :],
                        xsrc[bb * cfg.BLK:(bb + 1) * cfg.BLK, :],
                        idx_t[:, 8 * coff:8 * coff + nidx // 16],
                        num_idxs=nidx,
                        num_idxs_reg=nidx,
                        elem_size=F,
                        single_packet=False,
                        queue_num=bb,
                    )

                # one-hot A in [e, d, c] layout: dest/val broadcast on the
                # middle dim; all innermost dims stride-1 -> DVE 2x eligible
                a_t = apool.tile([128, 128, C_MAXT], BF, tag="a")
                dest_b = dv_t[:, 0, None, :].broadcast_to((128, 128, n_t))
                val_b = dv_t[:, 1, None, :].broadcast_to((128, 128, n_t))
                nc.vector.tensor_tensor(a_t[:, :, :n_t], iota_t[:, :, :n_t],
                                        dest_b, op=mybir.AluOpType.is_equal)
                nc.vector.tensor_tensor(a_t[:, :, :n_t], a_t[:, :, :n_t],
                                        val_b, op=mybir.AluOpType.mult)

                # segment-sum into PSUM
                e1 = e1pool.tile([128, F], FP, tag="e1")
                for c in range(n_t):
                    nc.tensor.matmul(
                        e1[:], a_t[:, :, c], xg[:, c, :],
                        start=(c == 0), stop=(c == n_t - 1),
                    )

                # epilogue of the PREVIOUS slot (so PE goes straight from
                # this slot's segment-sum into finished work)
                if prev is not None:
                    _epilogue(nc, cfg, prev, wpool, trpool, opool,
                              dscale_t, bias_t, ident_t, w_t, out_d)
                prev = (e1, xo, tt)
                toff += n_t

            _epilogue(nc, cfg, prev, wpool, trpool, opool,
                      dscale_t, bias_t, ident_t, w_t, out_d)

    nc.compile()
    return nc


def _epilogue(nc, cfg, prev, wpool, trpool, opool,
              dscale_t, bias_t, ident_t, w_t, out_d):
    F = cfg.F
    KC = F // 128
    e1, xo, tt = prev
    # e4 = xo - e1 * dscale   (bf16 result for the projection)
    t0 = wpool.tile([128, F], FP, tag="t0")
    nc.vector.tensor_tensor(t0[:], e1[:], dscale_t[:],
                            op=mybir.AluOpType.mult)
    e4 = wpool.tile([128, F], BF, tag="e4")
    nc.vector.tensor_tensor(e4[:], xo[:], t0[:],
                            op=mybir.AluOpType.subtract)

    # transpose e4 (PE), copy to SBUF on ACT
    ps_tr = trpool.tile([128, KC, 128], BF, tag="tr")
    for kc in range(KC):
        nc.tensor.transpose(ps_tr[:, kc, :],
                            e4[:, kc * 128:(kc + 1) * 128],
                            ident_t[:])
    e4T = wpool.tile([128, KC, 128], BF, tag="e4T")
    nc.scalar.copy(e4T[:], ps_tr[:])

    # out = e4 @ W + bias
    ps_out = opool.tile([128, F], FP, tag="po")
    for kc in range(KC):
        nc.tensor.matmul(ps_out[:], e4T[:, kc, :], w_t[:, kc, :],
                         start=(kc == 0), stop=(kc == KC - 1))
    outs = wpool.tile([128, F], BF, tag="outs")
    nc.vector.tensor_tensor(outs[:], ps_out[:], bias_t[:],
                            op=mybir.AluOpType.add)
    nc.sync.dma_start(out_d[tt * 128:(tt + 1) * 128, :], outs[:])


def _make_in_maps(cfg, c_maxt, x, weight, learnable_diag, bias,
                  idx_packed, dv_cols, tile_at):
    F, NC = cfg.F, cfg.NC
    x16 = x.astype(BF_NP)
    w_host = np.ascontiguousarray(
        weight.reshape(F // 128, 128, F).transpose(1, 0, 2)).astype(BF_NP)
    dscale_host = np.tile((learnable_diag + 1.0)[None, :], (128, 1))
    bias_host = np.tile(bias[None, :], (128, 1))
    iota_host = np.ascontiguousarray(np.broadcast_to(
        np.arange(128, dtype=np.float32).astype(BF_NP)[None, :, None],
        (128, 128, c_maxt)))
    ident_host = np.eye(128, dtype=np.float32).astype(BF_NP)

    # xown reordered by slot: slot s of core c holds tile tile_at[c, s]
    xpad = np.zeros((NC, cfg.PAD_ROWS, F), dtype=np.float32)
    xpad[:, :cfg.RPC, :] = x.reshape(NC, cfg.RPC, F)
    xown_slot = np.empty((NC, cfg.PAD_ROWS, F), dtype=BF_NP)
    for c in range(NC):
        xown_slot[c] = xpad[c].reshape(cfg.TILES, 128, F)[
            tile_at[c]].reshape(cfg.PAD_ROWS, F)

    in_maps = []
    for c in range(NC):
        in_maps.append({
            "xsrc": x16,
            "xown": xown_slot[c],
            "idx": np.ascontiguousarray(idx_packed[c]),
            "dv": np.ascontiguousarray(dv_cols[c]),
            "wt": w_host,
            "dscale": dscale_host,
            "bias": bias_host,
            "iota": iota_host,
            "ident": ident_host,
        })
    return in_maps


def run(cfg, x, edge_row, edge_col, edge_val, weight, learnable_diag, bias,
        trace_dir=None):
    x = np.ascontiguousarray(np.asarray(x, dtype=np.float32))
    weight = np.asarray(weight, dtype=np.float32)
    learnable_diag = np.asarray(learnable_diag, dtype=np.float32)
    bias = np.asarray(bias, dtype=np.float32)

    C2, NIDX2, pad_off, idx_packed, dv_cols, tile_at = _preprocess(
        cfg, edge_row, edge_col, edge_val)
    nc = _build(cfg, C2, NIDX2, pad_off)
    c_maxt = int(C2.sum(axis=1).max())
    in_maps = _make_in_maps(cfg, c_maxt, x, weight, learnable_diag, bias,
                            idx_packed, dv_cols, tile_at)

    kwargs = {}
    if trace_dir:
        kwargs = dict(trace=True, tmpdir=trace_dir)
    res = run_bass_kernel_spmd(nc, in_maps, core_ids=list(range(cfg.NC)),
                               **kwargs)
    out = np.empty((cfg.N, cfg.F), dtype=np.float32)
    for c in range(cfg.NC):
        o = res.results[c]["out"].astype(np.float32).reshape(
            cfg.TILES, 128, cfg.F)
        full = np.empty((cfg.TILES, 128, cfg.F), dtype=np.float32)
        full[tile_at[c]] = o                     # slot s -> tile tile_at[c,s]
        out[c * cfg.RPC:(c + 1) * cfg.RPC] = full.reshape(
            cfg.PAD_ROWS, cfg.F)[:cfg.RPC]
    return out, res


def kernel(x, edge_row, edge_col, edge_val, weight, learnable_diag, bias,
           _want_trace=None):
    cfg = Cfg()
    out, res = run(cfg, x, edge_row, edge_col, edge_val, weight,
                   learnable_diag, bias, trace_dir=_want_trace)
    kernel._last_results = res
    return out
